# revision 5
# baseline (speedup 1.0000x reference)
"""Chamfer distance loss kernel for Trainium2 (Bass/Tile), 8-core data parallel.

Problem: x, y [16, 2048, 3] fp32. Per batch b:
    P[i,j] = |x_i|^2 + |y_j|^2 - 2 x_i.y_j
    loss[b] = mean_j min_i P[i,j] + mean_i min_j P[i,j]

Strategy:
  - Shard batch dim: 2 batches per core across 8 cores, no cross-core comm.
  - P = -2*Q with Q[i,j] = x_i.y_j - 0.5|x_i|^2 - 0.5|y_j|^2 as a K=13 bf16
    double-split augmented matmul (h+m mantissa splits).  min P == -2 max Q.
  - Setup for BOTH batches runs combined up front (wide field ops, one DRAM
    round-trip store), so batch 1's setup is off the critical path.  Replicas
    for batch 0 load on the sync HWDGE queue, batch 1 via gpsimd SWDGE.
  - PE row tiling: per m-tile, 4 concurrent matmuls (tile_position=(32t,0))
    cover its four 512-wide n-chunks into one [128,2048] PSUM group.
  - ACT drains each group to bf16 SBUF; DVE runs the dl running-max chain
    and the dr row-max tree at 2x bf16 rate.
  - dl partition-axis max: batch 0 via gpsimd partition_all_reduce quarters
    (hidden under batch 1's main loop) + one fused ACT accum; batch 1 (the
    exposed tail) via DVE StreamTranspose + inner-32 reduce + 4 small
    32-channel gpsimd all-reduces + tiny TT folds.
"""

import sys

if "/opt/trn_rl_repo" not in sys.path:
    sys.path.insert(0, "/opt/trn_rl_repo")

import numpy as np

B, N, D = 16, 2048, 3
NCORES = 8
BPC = B // NCORES  # batches per core
MT = N // 128  # 16 m-tiles
Q = N // 128  # 16 points per partition in natural layout
K = 13

_CACHE = {}


def _build():
    from contextlib import ExitStack

    import concourse.bass as bass
    import concourse.mybir as mybir
    import concourse.tile as tile
    from concourse import bacc, bass_isa

    f32 = mybir.dt.float32
    bf16 = mybir.dt.bfloat16

    nc = bacc.Bacc()
    x = nc.dram_tensor("x", [BPC, N, D], f32, kind="ExternalInput")
    y = nc.dram_tensor("y", [BPC, N, D], f32, kind="ExternalInput")
    o = nc.dram_tensor("o", [1, BPC], f32, kind="ExternalOutput")

    X = mybir.AxisListType.X
    MAXOP = mybir.AluOpType.max

    with tile.TileContext(nc) as tc, ExitStack() as ctx:
        singles = ctx.enter_context(tc.tile_pool(name="singles", bufs=1))
        nat_pool = ctx.enter_context(tc.tile_pool(name="nat", bufs=1))
        aug_pool = ctx.enter_context(tc.tile_pool(name="aug", bufs=2))
        small_pool = ctx.enter_context(tc.tile_pool(name="small", bufs=4))
        run_pool = ctx.enter_context(tc.tile_pool(name="run", bufs=2))
        dr_pool = ctx.enter_context(tc.tile_pool(name="dr", bufs=2))
        cp_pool = ctx.enter_context(tc.tile_pool(name="cp", bufs=6))
        mm_psum = ctx.enter_context(tc.tile_pool(name="mmps", bufs=2, space="PSUM"))

        out_sb = singles.tile([1, BPC], f32)
        # scratch DRAM for the p->free flatten round trip, both batches
        scratch = nc.dram_tensor("scratch", [BPC, 2, K, 128, Q], bf16, kind="Internal")
        vscr = nc.dram_tensor("vscr", [128, 64], bf16, kind="Internal")

        # ---- combined setup: both batches, both sides ----
        # natxy [128, b, g, (q d)] f32
        natxy = singles.tile([128, BPC * 2 * Q * D], f32)
        natb = natxy.rearrange("p (b g qd) -> p b g qd", b=BPC, g=2)
        nc.sync.dma_start(
            out=natb[:, :, 0, :],
            in_=x[:, :, :].rearrange("b (p q) d -> p b (q d)", p=128),
        )
        nc.scalar.dma_start(
            out=natb[:, :, 1, :],
            in_=y[:, :, :].rearrange("b (p q) d -> p b (q d)", p=128),
        )
        # strided [p, b, g, d, q] view
        natv = natxy.rearrange("p (b g q d) -> p b g d q", b=BPC, g=2, d=D)

        # stv [128, b, g, f, q] bf16
        stxy = singles.tile([128, BPC * 2 * K * Q], bf16)
        stv = stxy.rearrange("p (b g f q) -> p b g f q", b=BPC, g=2, f=K)
        # x fields: [h0,h1,h2, h0,h1,h2, m0,m1,m2, nh,nm, 1,1]
        # y fields: [h0,h1,h2, m0,m1,m2, h0,h1,h2, 1,1, nh,nm]
        # h main: both sides, both batches, fields 0-2 in one op
        nc.vector.tensor_copy(stv[:, :, :, 0:3, :], natv)
        # h dup: x fields 3-5, y fields 6-8
        nc.scalar.copy(stv[:, :, 0, 3:6, :], natv[:, :, 0])
        nc.scalar.copy(stv[:, :, 1, 6:9, :], natv[:, :, 1])
        # residual m = nat - h
        tmp = singles.tile([128, BPC * 2 * Q * D], f32)
        tmpv = tmp.rearrange("p (b g d q) -> p b g d q", b=BPC, g=2, q=Q)
        nc.vector.tensor_sub(tmpv, natv, stv[:, :, :, 0:3, :])
        nc.scalar.copy(stv[:, :, 0, 6:9, :], tmpv[:, :, 0])
        nc.scalar.copy(stv[:, :, 1, 3:6, :], tmpv[:, :, 1])
        # norms: -0.5*|.|^2, split h+m
        sq = singles.tile([128, BPC * 2 * Q * D], f32)
        nc.vector.tensor_mul(sq, natxy, natxy)
        nrm = small_pool.tile([128, BPC * 2 * Q], f32, tag="nrm")
        nc.vector.tensor_reduce(
            nrm, sq.rearrange("p (bgq d) -> p bgq d", d=D), axis=X,
            op=mybir.AluOpType.add,
        )
        nc.vector.tensor_scalar_mul(nrm, nrm, -0.5)
        nrmv = nrm.rearrange("p (b g q) -> p b g q", b=BPC, g=2)
        nhx, nhy = stv[:, :, 0, 9, :], stv[:, :, 1, 11, :]
        nc.vector.tensor_copy(nhx, nrmv[:, :, 0])
        nc.vector.tensor_copy(nhy, nrmv[:, :, 1])
        nrm2 = small_pool.tile([128, BPC * 2 * Q], f32, tag="nrm2")
        nrm2v = nrm2.rearrange("p (b g q) -> p b g q", b=BPC, g=2)
        nc.vector.tensor_sub(nrm2v[:, :, 0], nrmv[:, :, 0], nhx)
        nc.vector.tensor_sub(nrm2v[:, :, 1], nrmv[:, :, 1], nhy)
        nc.scalar.copy(stv[:, :, 0, 10, :], nrm2v[:, :, 0])
        nc.scalar.copy(stv[:, :, 1, 12, :], nrm2v[:, :, 1])
        # ones fields: x f11:13, y f9:11 (strided over b)
        nc.gpsimd.memset(stv[:, :, 0, 11:13, :], 1.0)
        nc.gpsimd.memset(stv[:, :, 1, 9:11, :], 1.0)

        # one store for both batches: scratch[b,g,f,p,q] <- stv iter (p,b,g,f,q)
        nc.sync.dma_start(
            out=scratch[:, :, :, :, :].rearrange("b g f p q -> p b g f q"),
            in_=stv,
        )
        # replicas at partition blocks 0/32/64/96
        augs = []
        for b in range(BPC):
            aug_b = aug_pool.tile([128, 2 * N], bf16, tag="aug")
            augs.append(aug_b)
        for b in range(BPC):
            for r in range(4):
                eng = nc.sync
                eng.dma_start(
                    out=augs[b][32 * r : 32 * r + K].rearrange(
                        "f (g p q) -> f g p q", g=2, q=Q
                    ),
                    in_=scratch[b].rearrange("g f p q -> f g p q"),
                )

        dlsum_t = [None, None]

        for b in range(BPC):
            aug = augs[b]
            # ---- main loop: 16 quads of 4 concurrent matmuls ----
            runmax = run_pool.tile([128, N], bf16, tag="runmax")
            drbuf = dr_pool.tile([128, MT * 1024], bf16, tag="drbuf")
            for m in range(MT):
                psg = mm_psum.tile([128, 2048], f32, tag="mm")
                # m==0 uses only replica blocks 0/1 so it can start as soon
                # as the first two replica DMAs land
                rblk = (0, 0, 1, 1) if (b == 0 and m == 0) else (0, 1, 2, 3)
                for t in range(4):
                    blk = aug[32 * rblk[t] : 32 * rblk[t] + K]
                    nc.tensor.matmul(
                        psg[:, t * 512 : (t + 1) * 512],
                        lhsT=blk[:, m * 128 : (m + 1) * 128],
                        rhs=blk[:, N + t * 512 : N + (t + 1) * 512],
                        start=True,
                        stop=True,
                        tile_position=(32 * rblk[t], 0),
                    )
                if m % 2 == 0:
                    cpp = cp_pool.tile([128, 4096], bf16, tag="cpp")
                cp = cpp[:, (m % 2) * 2048 : (m % 2 + 1) * 2048]
                nc.scalar.copy(cp, psg)
                if m == 1:
                    nc.vector.tensor_max(
                        runmax, cpp[:, 0:2048], cpp[:, 2048:4096]
                    )
                elif m > 1:
                    nc.vector.tensor_max(runmax, runmax, cp)
                # dr level 1 batched per cp pair: fold column halves
                if m % 2 == 1:
                    cppv = cpp.rearrange("p (a c) -> p a c", a=2)
                    nc.vector.tensor_max(
                        drbuf[:, (m - 1) * 1024 : (m + 1) * 1024].rearrange(
                            "p (a c) -> p a c", a=2
                        ),
                        cppv[:, :, 0:1024],
                        cppv[:, :, 1024:2048],
                    )
                # tree levels folded into the loop (chunked, strided)
                drv = drbuf.rearrange("p (mm c) -> p mm c", c=1024)
                if m % 4 == 3:
                    s = slice(m - 3, m + 1)
                    nc.vector.tensor_max(
                        drv[:, s, 0:512], drv[:, s, 0:512], drv[:, s, 512:1024]
                    )
                if m % 8 == 7:
                    s = slice(m - 7, m + 1)
                    nc.vector.tensor_max(
                        drv[:, s, 0:256], drv[:, s, 0:256], drv[:, s, 256:512]
                    )
                    nc.vector.tensor_max(
                        drv[:, s, 0:128], drv[:, s, 0:128], drv[:, s, 128:256]
                    )
                    # final reduce for this half, still inside the loop
                    if m == 7:
                        dr16 = small_pool.tile([128, MT], f32, tag="dr16")
                    nc.vector.tensor_reduce(
                        dr16[:, m - 7 : m + 1], drv[:, s, 0:128], axis=X,
                        op=MAXOP,
                    )

            # ---- dl: partition-axis max ----
            dlsum = small_pool.tile([1, 1], f32, tag="dlsum")
            if b == 0:
                # hidden under batch 1's main loop: gpsimd quarters + 1 ACT
                dlall = run_pool.tile([128, N], bf16, tag="dlall")
                NQ = N // 4
                for qq in range(4):
                    sl = slice(qq * NQ, (qq + 1) * NQ)
                    nc.gpsimd.partition_all_reduce(
                        dlall[:, sl], runmax[:, sl], channels=128,
                        reduce_op=bass_isa.ReduceOp.max,
                    )
                junk = small_pool.tile([1, N], bf16, tag="junk")
                nc.scalar.activation(
                    junk, dlall[0:1, :],
                    mybir.ActivationFunctionType.Copy,
                    accum_out=dlsum,
                )
            else:
                # exposed tail: StreamTranspose route
                # Z[32a+i, 32c+j] = R[32a+j, 32c+i]
                zt = run_pool.tile([128, N], bf16, tag="dlall")
                nc.vector.transpose(zt, runmax)
                # V[(a,i), c] = max_j Z[(a,i), 32c+j]  (= group-a max of
                # column f=32c+i)
                vt = small_pool.tile([128, N // 32], bf16, tag="vt")
                nc.vector.tensor_reduce(
                    vt, zt.rearrange("p (c j) -> p c j", j=32), axis=X,
                    op=MAXOP,
                )
                # fold the 4 partition groups via a small DRAM hop:
                # vt [128, 64] -> DRAM -> wt [32, (a c)]
                nc.sync.dma_start(out=vscr[:, :], in_=vt)
                wt = small_pool.tile([32, 4 * 64], bf16, tag="wt")
                nc.sync.dma_start(
                    out=wt.rearrange("i (a c) -> i a c", a=4),
                    in_=vscr[:, :].rearrange("(a i) c -> i a c", a=4),
                )
                m01 = small_pool.tile([32, 64], f32, tag="m01")
                nc.vector.tensor_reduce(
                    m01, wt.rearrange("i (a c) -> i c a", a=4), axis=X,
                    op=MAXOP,
                )
                # sum all 2048 dl values: ACT accum per partition + 32-add
                dls32 = small_pool.tile([32, 1], f32, tag="dls32")
                junk2 = small_pool.tile([32, 64], bf16, tag="junk2")
                nc.scalar.activation(
                    junk2, m01,
                    mybir.ActivationFunctionType.Copy,
                    accum_out=dls32,
                )
                dlsr = small_pool.tile([32, 1], f32, tag="dlsr")
                nc.gpsimd.partition_all_reduce(
                    dlsr, dls32, channels=32,
                    reduce_op=bass_isa.ReduceOp.add,
                )
                nc.vector.tensor_copy(dlsum, dlsr[0:1, :])
            dlsum_t[b] = dlsum

            # ---- dr partition sum: all-reduce add, then combine ----
            drsum = small_pool.tile([128, 1], f32, tag="drsum")
            nc.vector.reduce_sum(drsum, dr16, axis=X)
            drall = small_pool.tile([128, 1], f32, tag="drall")
            nc.gpsimd.partition_all_reduce(
                drall, drsum, channels=128, reduce_op=bass_isa.ReduceOp.add
            )
            tot = small_pool.tile([1, 1], f32, tag="tot")
            nc.vector.tensor_add(tot, dlsum_t[b], drall[0:1, :])
            nc.vector.tensor_scalar_mul(out_sb[0:1, b : b + 1], tot, -2.0 / N)

        nc.sync.dma_start(out=o[0:1, 0:BPC], in_=out_sb)

    nc.compile()
    return nc


def _get_nc():
    if "nc" not in _CACHE:
        _CACHE["nc"] = _build()
    return _CACHE["nc"]


def kernel(x: np.ndarray, y: np.ndarray) -> np.ndarray:
    from concourse.bass_utils import run_bass_kernel_spmd

    x = np.ascontiguousarray(np.asarray(x, dtype=np.float32))
    y = np.ascontiguousarray(np.asarray(y, dtype=np.float32))
    nc = _get_nc()
    in_maps = [
        {"x": x[c * BPC : (c + 1) * BPC], "y": y[c * BPC : (c + 1) * BPC]}
        for c in range(NCORES)
    ]
    res = run_bass_kernel_spmd(nc, in_maps, core_ids=list(range(NCORES)))
    return np.concatenate([r["o"].reshape(BPC) for r in res.results])


# revision 6
# speedup vs baseline: 1.0341x; 1.0341x over previous
"""Chamfer distance loss kernel for Trainium2 (Bass/Tile), 8-core data parallel.

Problem: x, y [16, 2048, 3] fp32. Per batch b:
    P[i,j] = |x_i|^2 + |y_j|^2 - 2 x_i.y_j
    loss[b] = mean_j min_i P[i,j] + mean_i min_j P[i,j]

Strategy:
  - Shard batch dim: 2 batches per core across 8 cores, no cross-core comm.
  - P = -2*Q with Q[i,j] = x_i.y_j - 0.5|x_i|^2 - 0.5|y_j|^2 as a K=13 bf16
    double-split augmented matmul (h+m mantissa splits).  min P == -2 max Q.
  - Setup for BOTH batches is built with combined wide field ops up front.
    Batch 0's store/replica DMAs split across the sync+scalar HWDGE queues;
    batch 1's run on the idle sync queue, emitted mid-loop.
  - PE row tiling: per m-tile, 4 concurrent matmuls (tile_position=(32t,0))
    cover its four 512-wide n-chunks into one [128,2048] PSUM group.
  - ACT drains each group to bf16 SBUF; DVE runs the dl running-max chain
    and the dr row-max tree at 2x bf16 rate.
  - dl partition-axis max via gpsimd partition_all_reduce in quarters.  The
    final runmax TT is split into 4 column-quarter TTs so the gpsimd
    quarters start while DVE still drains its dr-tree backlog; one fused
    ACT accumulation per batch sums the dl row.
"""

import sys

if "/opt/trn_rl_repo" not in sys.path:
    sys.path.insert(0, "/opt/trn_rl_repo")

import numpy as np

B, N, D = 16, 2048, 3
NCORES = 8
BPC = B // NCORES  # batches per core
MT = N // 128  # 16 m-tiles
Q = N // 128  # 16 points per partition in natural layout
K = 13

_CACHE = {}


def _build():
    from contextlib import ExitStack

    import concourse.bass as bass
    import concourse.mybir as mybir
    import concourse.tile as tile
    from concourse import bacc, bass_isa

    f32 = mybir.dt.float32
    bf16 = mybir.dt.bfloat16

    nc = bacc.Bacc()
    x = nc.dram_tensor("x", [BPC, N, D], f32, kind="ExternalInput")
    y = nc.dram_tensor("y", [BPC, N, D], f32, kind="ExternalInput")
    o = nc.dram_tensor("o", [1, BPC], f32, kind="ExternalOutput")

    X = mybir.AxisListType.X
    MAXOP = mybir.AluOpType.max

    with tile.TileContext(nc) as tc, ExitStack() as ctx:
        singles = ctx.enter_context(tc.tile_pool(name="singles", bufs=1))
        aug_pool = ctx.enter_context(tc.tile_pool(name="aug", bufs=2))
        small_pool = ctx.enter_context(tc.tile_pool(name="small", bufs=4))
        run_pool = ctx.enter_context(tc.tile_pool(name="run", bufs=2))
        dr_pool = ctx.enter_context(tc.tile_pool(name="dr", bufs=2))
        cp_pool = ctx.enter_context(tc.tile_pool(name="cp", bufs=7))
        mm_psum = ctx.enter_context(tc.tile_pool(name="mmps", bufs=2, space="PSUM"))

        out_sb = singles.tile([1, BPC], f32)
        dlsums = singles.tile([1, BPC], f32)
        dralls = singles.tile([128, BPC], f32)
        tot2 = singles.tile([1, BPC], f32)
        scratch = nc.dram_tensor("scratch", [BPC, 2, K, 128, Q], bf16, kind="Internal")

        # ---- combined setup: both batches, both sides ----
        natxy = singles.tile([128, BPC * 2 * Q * D], f32)
        natb = natxy.rearrange("p (b g qd) -> p b g qd", b=BPC, g=2)
        nc.sync.dma_start(
            out=natb[:, :, 0, :],
            in_=x[:, :, :].rearrange("b (p q) d -> p b (q d)", p=128),
        )
        nc.scalar.dma_start(
            out=natb[:, :, 1, :],
            in_=y[:, :, :].rearrange("b (p q) d -> p b (q d)", p=128),
        )
        natv = natxy.rearrange("p (b g q d) -> p b g d q", b=BPC, g=2, d=D)

        stxy = singles.tile([128, BPC * 2 * K * Q], bf16)
        stv = stxy.rearrange("p (b g f q) -> p b g f q", b=BPC, g=2, f=K)
        # x fields: [h0,h1,h2, h0,h1,h2, m0,m1,m2, nh,nm, 1,1]
        # y fields: [h0,h1,h2, m0,m1,m2, h0,h1,h2, 1,1, nh,nm]
        nc.vector.tensor_copy(stv[:, :, :, 0:3, :], natv)
        nc.scalar.copy(stv[:, :, 0, 3:6, :], natv[:, :, 0])
        nc.scalar.copy(stv[:, :, 1, 6:9, :], natv[:, :, 1])
        tmp = singles.tile([128, BPC * 2 * Q * D], f32)
        tmpv = tmp.rearrange("p (b g d q) -> p b g d q", b=BPC, g=2, q=Q)
        nc.vector.tensor_sub(tmpv, natv, stv[:, :, :, 0:3, :])
        nc.scalar.copy(stv[:, :, 0, 6:9, :], tmpv[:, :, 0])
        nc.scalar.copy(stv[:, :, 1, 3:6, :], tmpv[:, :, 1])
        sq = singles.tile([128, BPC * 2 * Q * D], f32)
        nc.vector.tensor_mul(sq, natxy, natxy)
        nrm = small_pool.tile([128, BPC * 2 * Q], f32, tag="nrm")
        nc.vector.tensor_reduce(
            nrm, sq.rearrange("p (bgq d) -> p bgq d", d=D), axis=X,
            op=mybir.AluOpType.add,
        )
        nc.vector.tensor_scalar_mul(nrm, nrm, -0.5)
        nrmv = nrm.rearrange("p (b g q) -> p b g q", b=BPC, g=2)
        nhx, nhy = stv[:, :, 0, 9, :], stv[:, :, 1, 11, :]
        nc.vector.tensor_copy(nhx, nrmv[:, :, 0])
        nc.vector.tensor_copy(nhy, nrmv[:, :, 1])
        nrm2 = small_pool.tile([128, BPC * 2 * Q], f32, tag="nrm2")
        nrm2v = nrm2.rearrange("p (b g q) -> p b g q", b=BPC, g=2)
        nc.vector.tensor_sub(nrm2v[:, :, 0], nrmv[:, :, 0], nhx)
        nc.vector.tensor_sub(nrm2v[:, :, 1], nrmv[:, :, 1], nhy)
        nc.scalar.copy(stv[:, :, 0, 10, :], nrm2v[:, :, 0])
        nc.scalar.copy(stv[:, :, 1, 12, :], nrm2v[:, :, 1])
        nc.gpsimd.memset(stv[:, :, 0, 11:13, :], 1.0)
        nc.gpsimd.memset(stv[:, :, 1, 9:11, :], 1.0)

        augs = []
        for b in range(BPC):
            aug_b = aug_pool.tile([128, 2 * N], bf16, tag="aug")
            augs.append(aug_b)

        def emit_store(b, g, eng):
            eng.dma_start(
                out=scratch[b, g].rearrange("f p q -> p f q"),
                in_=stv[:, b, g],
            )

        def emit_replica(b, r, eng):
            eng.dma_start(
                out=augs[b][32 * r : 32 * r + K].rearrange(
                    "f (g p q) -> f g p q", g=2, q=Q
                ),
                in_=scratch[b].rearrange("g f p q -> f g p q"),
            )

        # batch 0 store + replicas split across the two HWDGE queues
        emit_store(0, 0, nc.sync)
        emit_store(0, 1, nc.scalar)
        emit_replica(0, 0, nc.sync)
        emit_replica(0, 1, nc.scalar)
        emit_replica(0, 2, nc.sync)
        emit_replica(0, 3, nc.scalar)

        for b in range(BPC):
            aug = augs[b]
            runmax = run_pool.tile([128, N], bf16, tag="runmax")
            drbuf = dr_pool.tile([128, MT * 1024], bf16, tag="drbuf")
            for m in range(MT):
                if b == 0 and m == 2:
                    # batch 1 setup DMAs on the idle sync queue, mid-loop
                    emit_store(1, 0, nc.sync)
                    emit_store(1, 1, nc.sync)
                    for r in range(4):
                        emit_replica(1, r, nc.sync)
                psg = mm_psum.tile([128, 2048], f32, tag="mm")
                rblk = (0, 0, 1, 1) if (b == 0 and m == 0) else (0, 1, 2, 3)
                for t in range(4):
                    blk = aug[32 * rblk[t] : 32 * rblk[t] + K]
                    nc.tensor.matmul(
                        psg[:, t * 512 : (t + 1) * 512],
                        lhsT=blk[:, m * 128 : (m + 1) * 128],
                        rhs=blk[:, N + t * 512 : N + (t + 1) * 512],
                        start=True,
                        stop=True,
                        tile_position=(32 * rblk[t], 0),
                    )
                if m % 2 == 0:
                    cpp = cp_pool.tile([128, 4096], bf16, tag="cpp")
                cp = cpp[:, (m % 2) * 2048 : (m % 2 + 1) * 2048]
                nc.scalar.copy(cp, psg)
                if m == 1:
                    nc.vector.tensor_max(
                        runmax, cpp[:, 0:2048], cpp[:, 2048:4096]
                    )
                elif m == MT - 1:
                    # final runmax split into column quarters so the gpsimd
                    # dl reduction can start while DVE drains its backlog
                    for qq in range(4):
                        sl = slice(qq * 512, (qq + 1) * 512)
                        nc.vector.tensor_max(
                            runmax[:, sl], runmax[:, sl], cp[:, sl]
                        )
                elif m > 1:
                    nc.vector.tensor_max(runmax, runmax, cp)
                # dr level 1 batched per cp pair: fold column halves
                if m % 2 == 1:
                    cppv = cpp.rearrange("p (a c) -> p a c", a=2)
                    nc.vector.tensor_max(
                        drbuf[:, (m - 1) * 1024 : (m + 1) * 1024].rearrange(
                            "p (a c) -> p a c", a=2
                        ),
                        cppv[:, :, 0:1024],
                        cppv[:, :, 1024:2048],
                    )
                drv = drbuf.rearrange("p (mm c) -> p mm c", c=1024)
                if m % 4 == 3:
                    s = slice(m - 3, m + 1)
                    nc.vector.tensor_max(
                        drv[:, s, 0:512], drv[:, s, 0:512], drv[:, s, 512:1024]
                    )
                if m % 8 == 7:
                    s = slice(m - 7, m + 1)
                    nc.vector.tensor_max(
                        drv[:, s, 0:256], drv[:, s, 0:256], drv[:, s, 256:512]
                    )
                    nc.vector.tensor_max(
                        drv[:, s, 0:128], drv[:, s, 0:128], drv[:, s, 128:256]
                    )
                    if m == 7:
                        dr16 = small_pool.tile([128, MT], f32, tag="dr16")
                    nc.vector.tensor_reduce(
                        dr16[:, m - 7 : m + 1], drv[:, s, 0:128], axis=X,
                        op=MAXOP,
                    )

            # ---- dl: partition-axis max via gpsimd quarters ----
            dlall = run_pool.tile([128, N], bf16, tag="dlall")
            NQ = N // 4
            for qq in range(4):
                sl = slice(qq * NQ, (qq + 1) * NQ)
                nc.gpsimd.partition_all_reduce(
                    dlall[:, sl], runmax[:, sl], channels=128,
                    reduce_op=bass_isa.ReduceOp.max,
                )
            junk = small_pool.tile([1, N], bf16, tag="junk")
            nc.scalar.activation(
                junk, dlall[0:1, :],
                mybir.ActivationFunctionType.Copy,
                accum_out=dlsums[0:1, b : b + 1],
            )

            # ---- dr partition sum ----
            drsum = small_pool.tile([128, 1], f32, tag="drsum")
            nc.vector.reduce_sum(drsum, dr16, axis=X)
            nc.gpsimd.partition_all_reduce(
                dralls[:, b : b + 1], drsum, channels=128,
                reduce_op=bass_isa.ReduceOp.add,
            )

        # ---- combine both batches in one pass ----
        nc.vector.tensor_add(tot2, dlsums, dralls[0:1, :])
        nc.vector.tensor_scalar_mul(out_sb, tot2, -2.0 / N)
        nc.sync.dma_start(out=o[0:1, 0:BPC], in_=out_sb)

    nc.compile()
    return nc


def _get_nc():
    if "nc" not in _CACHE:
        _CACHE["nc"] = _build()
    return _CACHE["nc"]


def kernel(x: np.ndarray, y: np.ndarray) -> np.ndarray:
    from concourse.bass_utils import run_bass_kernel_spmd

    x = np.ascontiguousarray(np.asarray(x, dtype=np.float32))
    y = np.ascontiguousarray(np.asarray(y, dtype=np.float32))
    nc = _get_nc()
    in_maps = [
        {"x": x[c * BPC : (c + 1) * BPC], "y": y[c * BPC : (c + 1) * BPC]}
        for c in range(NCORES)
    ]
    res = run_bass_kernel_spmd(nc, in_maps, core_ids=list(range(NCORES)))
    return np.concatenate([r["o"].reshape(BPC) for r in res.results])


# revision 7
# speedup vs baseline: 1.0488x; 1.0142x over previous
"""Chamfer distance loss kernel for Trainium2 (Bass/Tile), 8-core data parallel.

Problem: x, y [16, 2048, 3] fp32. Per batch b:
    P[i,j] = |x_i|^2 + |y_j|^2 - 2 x_i.y_j
    loss[b] = mean_j min_i P[i,j] + mean_i min_j P[i,j]

Strategy:
  - Shard batch dim: 2 batches per core across 8 cores, no cross-core comm.
  - P = -2*Q with Q[i,j] = x_i.y_j - 0.5|x_i|^2 - 0.5|y_j|^2 as a K=13 bf16
    double-split augmented matmul (h+m mantissa splits).  min P == -2 max Q.
  - Setup for BOTH batches is built with combined wide field ops up front.
    Batch 0's store/replica DMAs split across the sync+scalar HWDGE queues;
    batch 1's run on the idle sync queue, emitted mid-loop.
  - PE row tiling: per m-tile, 4 concurrent matmuls (tile_position=(32t,0))
    cover its four 512-wide n-chunks into one [128,2048] PSUM group.
  - ACT drains each group to bf16 SBUF; DVE runs the dl running-max chain
    and the dr row-max tree at 2x bf16 rate.
  - dl partition-axis max via gpsimd partition_all_reduce in quarters.  The
    final runmax TT is split into 4 column-quarter TTs so the gpsimd
    quarters start while DVE still drains its dr-tree backlog; one fused
    ACT accumulation per batch sums the dl row.
"""

import sys

if "/opt/trn_rl_repo" not in sys.path:
    sys.path.insert(0, "/opt/trn_rl_repo")

import numpy as np

B, N, D = 16, 2048, 3
NCORES = 8
BPC = B // NCORES  # batches per core
MT = N // 128  # 16 m-tiles
Q = N // 128  # 16 points per partition in natural layout
K = 13

_CACHE = {}


def _build():
    from contextlib import ExitStack

    import concourse.bass as bass
    import concourse.mybir as mybir
    import concourse.tile as tile
    from concourse import bacc, bass_isa

    f32 = mybir.dt.float32
    bf16 = mybir.dt.bfloat16

    nc = bacc.Bacc()
    x = nc.dram_tensor("x", [BPC, N, D], f32, kind="ExternalInput")
    y = nc.dram_tensor("y", [BPC, N, D], f32, kind="ExternalInput")
    o = nc.dram_tensor("o", [1, BPC], f32, kind="ExternalOutput")

    X = mybir.AxisListType.X
    MAXOP = mybir.AluOpType.max

    with tile.TileContext(nc) as tc, ExitStack() as ctx:
        singles = ctx.enter_context(tc.tile_pool(name="singles", bufs=1))
        aug_pool = ctx.enter_context(tc.tile_pool(name="aug", bufs=2))
        small_pool = ctx.enter_context(tc.tile_pool(name="small", bufs=4))
        run_pool = ctx.enter_context(tc.tile_pool(name="run", bufs=2))
        dr_pool = ctx.enter_context(tc.tile_pool(name="dr", bufs=2))
        cp_pool = ctx.enter_context(tc.tile_pool(name="cp", bufs=9))
        mm_psum = ctx.enter_context(tc.tile_pool(name="mmps", bufs=2, space="PSUM"))

        out_sb = singles.tile([1, BPC], f32)
        dlsums = singles.tile([1, BPC], f32)
        dralls = singles.tile([128, BPC], f32)
        tot2 = singles.tile([1, BPC], f32)
        scratch = nc.dram_tensor("scratch", [BPC, 2, K, 128, Q], bf16, kind="Internal")

        # ---- combined setup: both batches, both sides ----
        natxy = singles.tile([128, BPC * 2 * Q * D], f32)
        natb = natxy.rearrange("p (b g qd) -> p b g qd", b=BPC, g=2)
        nc.sync.dma_start(
            out=natb[:, :, 0, :],
            in_=x[:, :, :].rearrange("b (p q) d -> p b (q d)", p=128),
        )
        nc.scalar.dma_start(
            out=natb[:, :, 1, :],
            in_=y[:, :, :].rearrange("b (p q) d -> p b (q d)", p=128),
        )
        natv = natxy.rearrange("p (b g q d) -> p b g d q", b=BPC, g=2, d=D)

        stxy = singles.tile([128, BPC * 2 * K * Q], bf16)
        stv = stxy.rearrange("p (b g f q) -> p b g f q", b=BPC, g=2, f=K)
        # x fields: [h0,h1,h2, h0,h1,h2, m0,m1,m2, nh,nm, 1,1]
        # y fields: [h0,h1,h2, m0,m1,m2, h0,h1,h2, 1,1, nh,nm]
        nc.vector.tensor_copy(stv[:, :, :, 0:3, :], natv)
        nc.scalar.copy(stv[:, :, 0, 3:6, :], natv[:, :, 0])
        nc.scalar.copy(stv[:, :, 1, 6:9, :], natv[:, :, 1])
        tmp = singles.tile([128, BPC * 2 * Q * D], f32)
        tmpv = tmp.rearrange("p (b g d q) -> p b g d q", b=BPC, g=2, q=Q)
        nc.vector.tensor_sub(tmpv, natv, stv[:, :, :, 0:3, :])
        nc.scalar.copy(stv[:, :, 0, 6:9, :], tmpv[:, :, 0])
        nc.scalar.copy(stv[:, :, 1, 3:6, :], tmpv[:, :, 1])
        sq = singles.tile([128, BPC * 2 * Q * D], f32)
        nc.vector.tensor_mul(sq, natxy, natxy)
        nrm = small_pool.tile([128, BPC * 2 * Q], f32, tag="nrm")
        nc.vector.tensor_reduce(
            nrm, sq.rearrange("p (bgq d) -> p bgq d", d=D), axis=X,
            op=mybir.AluOpType.add,
        )
        nc.vector.tensor_scalar_mul(nrm, nrm, -0.5)
        nrmv = nrm.rearrange("p (b g q) -> p b g q", b=BPC, g=2)
        nhx, nhy = stv[:, :, 0, 9, :], stv[:, :, 1, 11, :]
        nc.vector.tensor_copy(nhx, nrmv[:, :, 0])
        nc.vector.tensor_copy(nhy, nrmv[:, :, 1])
        nrm2 = small_pool.tile([128, BPC * 2 * Q], f32, tag="nrm2")
        nrm2v = nrm2.rearrange("p (b g q) -> p b g q", b=BPC, g=2)
        nc.vector.tensor_sub(nrm2v[:, :, 0], nrmv[:, :, 0], nhx)
        nc.vector.tensor_sub(nrm2v[:, :, 1], nrmv[:, :, 1], nhy)
        nc.scalar.copy(stv[:, :, 0, 10, :], nrm2v[:, :, 0])
        nc.scalar.copy(stv[:, :, 1, 12, :], nrm2v[:, :, 1])
        nc.gpsimd.memset(stv[:, :, 0, 11:13, :], 1.0)
        nc.gpsimd.memset(stv[:, :, 1, 9:11, :], 1.0)

        augs = []
        for b in range(BPC):
            aug_b = aug_pool.tile([128, 2 * N], bf16, tag="aug")
            augs.append(aug_b)

        def emit_store(b, g, eng):
            eng.dma_start(
                out=scratch[b, g].rearrange("f p q -> p f q"),
                in_=stv[:, b, g],
            )

        def emit_replica(b, r, eng, g=None):
            if g is None:
                eng.dma_start(
                    out=augs[b][32 * r : 32 * r + K].rearrange(
                        "f (g p q) -> f g p q", g=2, q=Q
                    ),
                    in_=scratch[b].rearrange("g f p q -> f g p q"),
                )
            else:
                eng.dma_start(
                    out=augs[b][32 * r : 32 * r + K, g * N : (g + 1) * N].rearrange(
                        "f (p q) -> f p q", q=Q
                    ),
                    in_=scratch[b, g].rearrange("f p q -> f p q"),
                )

        # batch 0 store + replicas split across the two HWDGE queues;
        # replicas 0/1 split by side so each only waits on its own store
        emit_store(0, 0, nc.sync)
        emit_store(0, 1, nc.scalar)
        emit_replica(0, 0, nc.sync, g=0)
        emit_replica(0, 1, nc.scalar, g=1)
        emit_replica(0, 0, nc.scalar, g=1)
        emit_replica(0, 1, nc.sync, g=0)
        emit_replica(0, 2, nc.sync)
        emit_replica(0, 3, nc.scalar)

        for b in range(BPC):
            aug = augs[b]
            runmax = run_pool.tile([128, N], bf16, tag="runmax")
            drbuf = dr_pool.tile([128, MT * 1024], bf16, tag="drbuf")
            for m in range(MT):
                if b == 0 and m == 2:
                    # batch 1 setup DMAs on the idle sync queue, mid-loop
                    emit_store(1, 0, nc.sync)
                    emit_store(1, 1, nc.sync)
                    for r in range(4):
                        emit_replica(1, r, nc.sync)
                psg = mm_psum.tile([128, 2048], f32, tag="mm")
                rblk = (0, 0, 1, 1) if (b == 0 and m == 0) else (0, 1, 2, 3)
                for t in range(4):
                    blk = aug[32 * rblk[t] : 32 * rblk[t] + K]
                    nc.tensor.matmul(
                        psg[:, t * 512 : (t + 1) * 512],
                        lhsT=blk[:, m * 128 : (m + 1) * 128],
                        rhs=blk[:, N + t * 512 : N + (t + 1) * 512],
                        start=True,
                        stop=True,
                        tile_position=(32 * rblk[t], 0),
                    )
                if m % 2 == 0:
                    cpp = cp_pool.tile([128, 4096], bf16, tag="cpp")
                cp = cpp[:, (m % 2) * 2048 : (m % 2 + 1) * 2048]
                nc.scalar.copy(cp, psg)
                if m == 1:
                    nc.vector.tensor_max(
                        runmax, cpp[:, 0:2048], cpp[:, 2048:4096]
                    )
                elif m == MT - 1:
                    # final runmax split into column quarters so the gpsimd
                    # dl reduction can start while DVE drains its backlog
                    for qq in range(4):
                        sl = slice(qq * 512, (qq + 1) * 512)
                        nc.vector.tensor_max(
                            runmax[:, sl], runmax[:, sl], cp[:, sl]
                        )
                elif m > 1:
                    nc.vector.tensor_max(runmax, runmax, cp)
                # dr level 1 batched per cp pair: fold column halves
                if m % 2 == 1:
                    cppv = cpp.rearrange("p (a c) -> p a c", a=2)
                    nc.vector.tensor_max(
                        drbuf[:, (m - 1) * 1024 : (m + 1) * 1024].rearrange(
                            "p (a c) -> p a c", a=2
                        ),
                        cppv[:, :, 0:1024],
                        cppv[:, :, 1024:2048],
                    )
                drv = drbuf.rearrange("p (mm c) -> p mm c", c=1024)
                if m % 4 == 3:
                    s = slice(m - 3, m + 1)
                    nc.vector.tensor_max(
                        drv[:, s, 0:512], drv[:, s, 0:512], drv[:, s, 512:1024]
                    )
                if m % 8 == 7:
                    s = slice(m - 7, m + 1)
                    nc.vector.tensor_max(
                        drv[:, s, 0:256], drv[:, s, 0:256], drv[:, s, 256:512]
                    )
                    nc.vector.tensor_max(
                        drv[:, s, 0:128], drv[:, s, 0:128], drv[:, s, 128:256]
                    )
                    if m == 7:
                        dr16 = small_pool.tile([128, MT], f32, tag="dr16")
                    nc.vector.tensor_reduce(
                        dr16[:, m - 7 : m + 1], drv[:, s, 0:128], axis=X,
                        op=MAXOP,
                    )

            # ---- dl: partition-axis max via gpsimd quarters, ACT accum
            # pipelined per quarter ----
            dlall = run_pool.tile([128, N], bf16, tag="dlall")
            dlq = small_pool.tile([1, 4], f32, tag="dlq")
            junk = small_pool.tile([1, N // 4], bf16, tag="junk")
            NQ = N // 4
            for qq in range(4):
                sl = slice(qq * NQ, (qq + 1) * NQ)
                nc.gpsimd.partition_all_reduce(
                    dlall[:, sl], runmax[:, sl], channels=128,
                    reduce_op=bass_isa.ReduceOp.max,
                )
                nc.scalar.activation(
                    junk, dlall[0:1, sl],
                    mybir.ActivationFunctionType.Copy,
                    accum_out=dlq[0:1, qq : qq + 1],
                )
            nc.vector.reduce_sum(dlsums[0:1, b : b + 1], dlq, axis=X)

            # ---- dr partition sum ----
            drsum = small_pool.tile([128, 1], f32, tag="drsum")
            nc.vector.reduce_sum(drsum, dr16, axis=X)
            nc.gpsimd.partition_all_reduce(
                dralls[:, b : b + 1], drsum, channels=128,
                reduce_op=bass_isa.ReduceOp.add,
            )

        # ---- combine both batches in one pass ----
        nc.vector.tensor_add(tot2, dlsums, dralls[0:1, :])
        nc.vector.tensor_scalar_mul(out_sb, tot2, -2.0 / N)
        nc.sync.dma_start(out=o[0:1, 0:BPC], in_=out_sb)

    nc.compile()
    return nc


def _get_nc():
    if "nc" not in _CACHE:
        _CACHE["nc"] = _build()
    return _CACHE["nc"]


def kernel(x: np.ndarray, y: np.ndarray) -> np.ndarray:
    from concourse.bass_utils import run_bass_kernel_spmd

    x = np.ascontiguousarray(np.asarray(x, dtype=np.float32))
    y = np.ascontiguousarray(np.asarray(y, dtype=np.float32))
    nc = _get_nc()
    in_maps = [
        {"x": x[c * BPC : (c + 1) * BPC], "y": y[c * BPC : (c + 1) * BPC]}
        for c in range(NCORES)
    ]
    res = run_bass_kernel_spmd(nc, in_maps, core_ids=list(range(NCORES)))
    return np.concatenate([r["o"].reshape(BPC) for r in res.results])


# revision 8
# speedup vs baseline: 1.0594x; 1.0102x over previous
"""Chamfer distance loss kernel for Trainium2 (Bass/Tile), 8-core data parallel.

Problem: x, y [16, 2048, 3] fp32. Per batch b:
    P[i,j] = |x_i|^2 + |y_j|^2 - 2 x_i.y_j
    loss[b] = mean_j min_i P[i,j] + mean_i min_j P[i,j]

Strategy:
  - Shard batch dim: 2 batches per core across 8 cores, no cross-core comm.
  - P = -2*Q with Q[i,j] = x_i.y_j - 0.5|x_i|^2 - 0.5|y_j|^2 as a K=13 bf16
    double-split augmented matmul (h+m mantissa splits; dropped m*m terms
    ~2^-18 relative).  min P == -2 * max Q.
  - Setup: fused x+y field builds, flattened to [13, 2N] operand layout via
    one DRAM round trip, replicated at partition blocks 0/32/64/96.
  - PE row tiling: per m-tile, 4 concurrent matmuls (tile_position=(32t,0))
    cover its four 512-wide n-chunks into one [128,2048] PSUM group (all 8
    banks double-buffered), ~3x PE throughput vs serial.
  - ACT drains each group to bf16 SBUF; DVE runs the dl running-max chain
    and the dr row-max tree at its 2x bf16 rate (tree levels chunked into
    the loop so the batch tail stays short).
  - dl partition-axis max via gpsimd partition_all_reduce(max) in quarters,
    with the dl mean accumulating on ACT; the final runmax TT is split into
    4 column-quarter TTs so the gpsimd quarters overlap DVE's dr-tree
    backlog.  dr partition sum via partition_all_reduce(add).
"""

import sys

if "/opt/trn_rl_repo" not in sys.path:
    sys.path.insert(0, "/opt/trn_rl_repo")

import numpy as np

B, N, D = 16, 2048, 3
NCORES = 8
BPC = B // NCORES  # batches per core
MT = N // 128  # 16 m-tiles
Q = N // 128  # 16 points per partition in natural layout
K = 13

_CACHE = {}


def _build():
    from contextlib import ExitStack

    import concourse.bass as bass
    import concourse.mybir as mybir
    import concourse.tile as tile
    from concourse import bacc

    f32 = mybir.dt.float32
    bf16 = mybir.dt.bfloat16

    nc = bacc.Bacc()
    x = nc.dram_tensor("x", [BPC, N, D], f32, kind="ExternalInput")
    y = nc.dram_tensor("y", [BPC, N, D], f32, kind="ExternalInput")
    o = nc.dram_tensor("o", [1, BPC], f32, kind="ExternalOutput")

    X = mybir.AxisListType.X
    MAXOP = mybir.AluOpType.max

    with tile.TileContext(nc) as tc, ExitStack() as ctx:
        singles = ctx.enter_context(tc.tile_pool(name="singles", bufs=1))
        nat_pool = ctx.enter_context(tc.tile_pool(name="nat", bufs=3))
        stage_pool = ctx.enter_context(tc.tile_pool(name="stage", bufs=3))
        aug_pool = ctx.enter_context(tc.tile_pool(name="aug", bufs=3))
        small_pool = ctx.enter_context(tc.tile_pool(name="small", bufs=3))
        run_pool = ctx.enter_context(tc.tile_pool(name="run", bufs=2))
        dr_pool = ctx.enter_context(tc.tile_pool(name="dr", bufs=2))
        cp_pool = ctx.enter_context(tc.tile_pool(name="cp", bufs=9))
        mm_psum = ctx.enter_context(tc.tile_pool(name="mmps", bufs=2, space="PSUM"))

        out_sb = singles.tile([1, BPC], f32)
        scratch = nc.dram_tensor("scratch", [BPC, 2, K, 128, Q], bf16, kind="Internal")

        for b in range(BPC):
            # ---- setup: load both sides, split, flatten via DRAM ----
            natxy = nat_pool.tile([128, 2 * Q * D], f32, tag="natxy")
            nc.sync.dma_start(
                out=natxy[:, 0 : Q * D],
                in_=x[b].rearrange("(p q) d -> p (q d)", p=128),
            )
            nc.scalar.dma_start(
                out=natxy[:, Q * D : 2 * Q * D],
                in_=y[b].rearrange("(p q) d -> p (q d)", p=128),
            )
            # [p, g, d, q] strided view
            natv = natxy.rearrange("p (g q d) -> p g d q", g=2, d=D)

            stxy = stage_pool.tile([128, 2 * K * Q], bf16, tag="stxy")
            stv = stxy.rearrange("p (g f q) -> p g f q", g=2, f=K)
            # x fields: [h0,h1,h2, h0,h1,h2, m0,m1,m2, nh,nm, 1,1]
            # y fields: [h0,h1,h2, m0,m1,m2, h0,h1,h2, 1,1, nh,nm]
            # h main: both sides fields 0-2 in one op
            nc.vector.tensor_copy(stv[:, :, 0:3, :], natv)
            # h dup: x fields 3-5, y fields 6-8 (idle gpsimd)
            nc.scalar.copy(stv[:, 0, 3:6, :], natv[:, 0])
            nc.scalar.copy(stv[:, 1, 6:9, :], natv[:, 1])
            # residual m = nat - h (both sides, one op)
            tmp = nat_pool.tile([128, 2 * Q * D], f32, tag="tmp")
            tmpv = tmp.rearrange("p (g d q) -> p g d q", g=2, q=Q)
            nc.vector.tensor_sub(tmpv, natv, stv[:, :, 0:3, :])
            nc.scalar.copy(stv[:, 0, 6:9, :], tmpv[:, 0])
            nc.scalar.copy(stv[:, 1, 3:6, :], tmpv[:, 1])
            # norms: -0.5*|.|^2, split h+m
            sq = nat_pool.tile([128, 2 * Q * D], f32, tag="sq")
            nc.vector.tensor_mul(sq, natxy, natxy)
            nrm = small_pool.tile([128, 2 * Q], f32, tag="nrm")
            nc.vector.tensor_reduce(
                nrm, sq.rearrange("p (g q d) -> p g q d", g=2, d=D), axis=X,
                op=mybir.AluOpType.add,
            )
            nc.vector.tensor_scalar_mul(nrm, nrm, -0.5)
            nrmv = nrm.rearrange("p (g q) -> p g q", g=2)
            nhx, nhy = stv[:, 0, 9, :], stv[:, 1, 11, :]
            nc.vector.tensor_copy(nhx, nrmv[:, 0])
            nc.vector.tensor_copy(nhy, nrmv[:, 1])
            nrm2 = small_pool.tile([128, 2 * Q], f32, tag="nrm2")
            nrm2v = nrm2.rearrange("p (g q) -> p g q", g=2)
            nc.vector.tensor_sub(nrm2v[:, 0], nrmv[:, 0], nhx)
            nc.vector.tensor_sub(nrm2v[:, 1], nrmv[:, 1], nhy)
            nc.scalar.copy(stv[:, 0, 10, :], nrm2v[:, 0])
            nc.scalar.copy(stv[:, 1, 12, :], nrm2v[:, 1])
            # ones fields
            nc.gpsimd.memset(stxy[:, 11 * Q : 13 * Q], 1.0)
            nc.gpsimd.memset(stxy[:, (K + 9) * Q : (K + 11) * Q], 1.0)

            nc.sync.dma_start(
                out=scratch[b].rearrange("g f p q -> p g f q"),
                in_=stv,
            )
            # aug replicated at partition blocks 0/32/64/96 for PE row tiling
            aug = aug_pool.tile([128, 2 * N], bf16, tag="aug")
            for r in range(4):
                eng = nc.sync if r % 2 == 0 else nc.scalar
                eng.dma_start(
                    out=aug[32 * r : 32 * r + K].rearrange(
                        "f (g p q) -> f g p q", g=2, q=Q
                    ),
                    in_=scratch[b].rearrange("g f p q -> f g p q"),
                )

            # ---- main loop: 16 quads of 4 concurrent matmuls ----
            runmax = run_pool.tile([128, N], bf16, tag="runmax")
            drbuf = dr_pool.tile([128, MT * 1024], bf16, tag="drbuf")
            for m in range(MT):
                psg = mm_psum.tile([128, 2048], f32, tag="mm")
                # m==0 uses only replica blocks 0/1 so it can start as soon
                # as the first two replica DMAs land (pairs serialize within
                # a row group, the two groups run concurrently)
                rblk = (0, 0, 1, 1) if m == 0 else (0, 1, 2, 3)
                for t in range(4):
                    blk = aug[32 * rblk[t] : 32 * rblk[t] + K]
                    nc.tensor.matmul(
                        psg[:, t * 512 : (t + 1) * 512],
                        lhsT=blk[:, m * 128 : (m + 1) * 128],
                        rhs=blk[:, N + t * 512 : N + (t + 1) * 512],
                        start=True,
                        stop=True,
                        tile_position=(32 * rblk[t], 0),
                    )
                if m % 2 == 0:
                    cpp = cp_pool.tile([128, 4096], bf16, tag="cpp")
                cp = cpp[:, (m % 2) * 2048 : (m % 2 + 1) * 2048]
                nc.scalar.copy(cp, psg)
                if m == 1:
                    nc.vector.tensor_max(
                        runmax, cpp[:, 0:2048], cpp[:, 2048:4096]
                    )
                elif m == MT - 1:
                    # final runmax split into column quarters so the gpsimd
                    # dl reduction starts while DVE drains its backlog
                    for qq in range(4):
                        sl = slice(qq * 512, (qq + 1) * 512)
                        nc.vector.tensor_max(
                            runmax[:, sl], runmax[:, sl], cp[:, sl]
                        )
                elif m > 1:
                    nc.vector.tensor_max(runmax, runmax, cp)
                # dr level 1 batched per cp pair: fold column halves
                if m % 2 == 1:
                    cppv = cpp.rearrange("p (a c) -> p a c", a=2)
                    nc.vector.tensor_max(
                        drbuf[:, (m - 1) * 1024 : (m + 1) * 1024].rearrange(
                            "p (a c) -> p a c", a=2
                        ),
                        cppv[:, :, 0:1024],
                        cppv[:, :, 1024:2048],
                    )
                # tree levels folded into the loop (chunked, strided)
                drv = drbuf.rearrange("p (mm c) -> p mm c", c=1024)
                if m % 4 == 3:
                    s = slice(m - 3, m + 1)
                    nc.vector.tensor_max(
                        drv[:, s, 0:512], drv[:, s, 0:512], drv[:, s, 512:1024]
                    )
                if m % 8 == 7:
                    s = slice(m - 7, m + 1)
                    nc.vector.tensor_max(
                        drv[:, s, 0:256], drv[:, s, 0:256], drv[:, s, 256:512]
                    )
                    nc.vector.tensor_max(
                        drv[:, s, 0:128], drv[:, s, 0:128], drv[:, s, 128:256]
                    )
                    # final reduce for this half, still inside the loop
                    if m == 7:
                        dr16 = small_pool.tile([128, MT], f32, tag="dr16")
                    nc.vector.tensor_reduce(
                        dr16[:, m - 7 : m + 1], drv[:, s, 0:128], axis=X,
                        op=MAXOP,
                    )

            # ---- dl: partition-axis max on the idle gpsimd engine ----
            # quartered; dl sums accumulate on the scalar engine
            from concourse import bass_isa

            dlall = run_pool.tile([128, N], bf16, tag="dlall")
            dlq = small_pool.tile([1, 4], f32, tag="dlq")
            junk = small_pool.tile([1, N // 4], bf16, tag="junk")
            NQ = N // 4
            for qq in range(4):
                sl = slice(qq * NQ, (qq + 1) * NQ)
                nc.gpsimd.partition_all_reduce(
                    dlall[:, sl], runmax[:, sl], channels=128,
                    reduce_op=bass_isa.ReduceOp.max,
                )
                nc.scalar.activation(
                    junk, dlall[0:1, sl],
                    mybir.ActivationFunctionType.Copy,
                    accum_out=dlq[0:1, qq : qq + 1],
                )
            dlsum = small_pool.tile([1, 1], f32, tag="dlsum")
            nc.vector.reduce_sum(dlsum, dlq, axis=X)

            # ---- dr partition sum: all-reduce add, then combine ----
            drsum = small_pool.tile([128, 1], f32, tag="drsum")
            nc.vector.reduce_sum(drsum, dr16, axis=X)
            drall = small_pool.tile([128, 1], f32, tag="drall")
            nc.gpsimd.partition_all_reduce(
                drall, drsum, channels=128, reduce_op=bass_isa.ReduceOp.add
            )
            tot = small_pool.tile([1, 1], f32, tag="tot")
            nc.vector.tensor_add(tot, dlsum, drall[0:1, :])
            nc.vector.tensor_scalar_mul(out_sb[0:1, b : b + 1], tot, -2.0 / N)

        nc.sync.dma_start(out=o[0:1, 0:BPC], in_=out_sb)

    nc.compile()
    return nc


def _get_nc():
    if "nc" not in _CACHE:
        _CACHE["nc"] = _build()
    return _CACHE["nc"]


def kernel(x: np.ndarray, y: np.ndarray) -> np.ndarray:
    from concourse.bass_utils import run_bass_kernel_spmd

    x = np.ascontiguousarray(np.asarray(x, dtype=np.float32))
    y = np.ascontiguousarray(np.asarray(y, dtype=np.float32))
    nc = _get_nc()
    in_maps = [
        {"x": x[c * BPC : (c + 1) * BPC], "y": y[c * BPC : (c + 1) * BPC]}
        for c in range(NCORES)
    ]
    res = run_bass_kernel_spmd(nc, in_maps, core_ids=list(range(NCORES)))
    return np.concatenate([r["o"].reshape(BPC) for r in res.results])
